# revision 1
# baseline (speedup 1.0000x reference)
"""GAT layer kernel for Trainium2, sharded across 8 NeuronCores.

Math: since adj is 0/1 and the attention logit e_i is constant across row i,
the masked softmax collapses to attention[i,j] = adj[i,j] / rowdeg(i), so

    out = elu((adj @ h) / d),   h = x @ W,   d = adj @ ones

Per-core strategy (core c owns destination rows R_c = [c*1536, (c+1)*1536)):
  - host passes adjT_c = adj[R_c, :].T  (layout-only prep, [12288, 1536] int32)
  - device computes full h once (x replicated), augmented with a ones column
    -> h_aug [12288, 65] bf16 in SBUF
  - main loop over 96 k-blocks: SWDGE DMA-casts adjT block [128, 1536]
    int32->bf16, then PE accumulates s_aug^T[65, 1536] += h_aug[kb].T @ adjT
  - epilogue: PE-transpose s_aug^T back to row-major blocks, divide by the
    degree column, apply ELU, store [1536, 64] f32.
The adj traffic (75.5 MB int32 per core) is the memory roofline.
"""

import numpy as np

_N = 12288
_P = 128
_NCORES = 8
_ROWS = _N // _NCORES          # 1536 destination rows per core
_KB = _N // _P                 # 96 k-blocks
_INF = 256
_OUTF = 64
_HA = _OUTF + 1                # h augmented with ones column
_MT = _ROWS // 512             # 3 moving-operand tiles per k-block

_cached_nc = None
last_results = None            # BassKernelResults of the most recent run


def _build_nc():
    from contextlib import ExitStack

    import concourse.bacc as bacc
    import concourse.mybir as mybir
    import concourse.tile as tile
    from concourse.masks import make_identity

    f32 = mybir.dt.float32
    bf16 = mybir.dt.bfloat16
    ACT = mybir.ActivationFunctionType

    nc = bacc.Bacc("TRN2", target_bir_lowering=False, debug=False)
    adjT = nc.dram_tensor("adjT", [_N, _ROWS], mybir.dt.int32, kind="ExternalInput")
    xT = nc.dram_tensor("xT", [_INF, _N], f32, kind="ExternalInput")
    W = nc.dram_tensor("W", [_INF, _OUTF], f32, kind="ExternalInput")
    # raw staging layout [partition, t*64+f]; host reassembles rows as
    # out[t*128+p, f] = out_raw[p, t*64+f]. Keeps the store at 3KB/partition
    # contiguous chunks (a [1536, 64] row-major store would be 256B chunks,
    # under the 512B line-rate minimum -> RMW-slow).
    out = nc.dram_tensor("out", [_P, (_ROWS // _P) * _OUTF], f32,
                         kind="ExternalOutput")

    with ExitStack() as ctx:
        tc = ctx.enter_context(tile.TileContext(nc))
        cpool = ctx.enter_context(tc.tile_pool(name="cpool", bufs=1))
        xpool = ctx.enter_context(tc.tile_pool(name="xpool", bufs=1))
        hpool = ctx.enter_context(tc.tile_pool(name="hpool", bufs=1))
        apool = ctx.enter_context(tc.tile_pool(name="apool", bufs=22))
        epool = ctx.enter_context(tc.tile_pool(name="epool", bufs=4))
        ps_main = ctx.enter_context(tc.tile_pool(name="ps_main", bufs=1, space="PSUM"))
        ps_h = ctx.enter_context(tc.tile_pool(name="ps_h", bufs=1, space="PSUM"))
        ps_t = ctx.enter_context(tc.tile_pool(name="ps_t", bufs=3, space="PSUM"))

        ident = cpool.tile([_P, _P], f32, name="ident", tag="ident")
        make_identity(nc, ident[:])

        # W and xT loaded with SWDGE cast f32->bf16 (HBM reads unchanged,
        # but the h-phase matmuls become single-pass bf16 instead of the
        # 2x LOW_HIGH fp32 path)
        w_sb = cpool.tile([_P, 2 * _OUTF], bf16, name="w_sb", tag="w_sb")
        nc.gpsimd.dma_start(w_sb[:, 0:_OUTF], W[0:_P, :])
        nc.gpsimd.dma_start(w_sb[:, _OUTF:], W[_P:, :])

        xt0 = xpool.tile([_P, _N], bf16, name="xt0", tag="xt0")
        nc.gpsimd.dma_start(xt0[:], xT[0:_P, :])
        xt1 = xpool.tile([_P, _N], bf16, name="xt1", tag="xt1")
        nc.gpsimd.dma_start(xt1[:], xT[_P:, :])

        # h_aug blocks side by side: block ib occupies cols [ib*65, ib*65+65);
        # col 64 of each block is the ones column (memset once, never rewritten)
        h_aug = hpool.tile([_P, _KB * _HA], bf16, name="h_aug", tag="h_aug")
        nc.gpsimd.memset(h_aug[:], 1.0)

        for ib in range(_KB):
            ph = ps_h.tile([_P, _OUTF], f32, name="ph", tag="ph")
            nc.tensor.matmul(ph[:], lhsT=xt0[:, ib * _P:(ib + 1) * _P],
                             rhs=w_sb[:, 0:_OUTF], start=True, stop=False)
            nc.tensor.matmul(ph[:], lhsT=xt1[:, ib * _P:(ib + 1) * _P],
                             rhs=w_sb[:, _OUTF:], start=False, stop=True)
            nc.scalar.activation(h_aug[:, ib * _HA: ib * _HA + _OUTF], ph[:], ACT.Copy)

        # main accumulation: s_aug^T[f, m] += sum_kb h_aug[kb].T @ adjT[kb]
        ps = ps_main.tile([_HA, _ROWS], f32, name="ps", tag="ps")
        for kb in range(_KB):
            at = apool.tile([_P, _ROWS], bf16, name="at", tag="at")
            nc.gpsimd.dma_start(at[:], adjT[kb * _P:(kb + 1) * _P, :])
            for mt in range(_MT):
                nc.tensor.matmul(
                    ps[:, mt * 512:(mt + 1) * 512],
                    lhsT=h_aug[:, kb * _HA:(kb + 1) * _HA],
                    rhs=at[:, mt * 512:(mt + 1) * 512],
                    start=(kb == 0), stop=(kb == _KB - 1),
                )

        # epilogue: transpose back, normalize by degree, ELU; stage all 12
        # row-blocks into one SBUF tile and store with a single DMA
        out_stage = hpool.tile([_P, (_ROWS // _P) * _OUTF], f32,
                               name="out_stage", tag="out_stage")
        for t in range(_ROWS // _P):
            sT = epool.tile([_HA, _P], f32, name="sT", tag="sT")
            nc.scalar.activation(sT[:], ps[:, t * _P:(t + 1) * _P], ACT.Copy)
            tp = ps_t.tile([_P, _HA], f32, name="tp", tag="tp")
            nc.tensor.transpose(tp[:], sT[:], ident[0:_HA, 0:_HA])
            rec = epool.tile([_P, 1], f32, name="rec", tag="rec")
            nc.vector.reciprocal(rec[:], tp[:, _OUTF:_HA])
            z = epool.tile([_P, _OUTF], f32, name="z", tag="z")
            nc.vector.tensor_scalar_mul(z[:], tp[:, 0:_OUTF], rec[:])
            # elu(z) = relu(z) - relu(1 - exp(z)): exact both branches, and
            # two ops shorter than the min/exp/max/add/sub chain
            ex = epool.tile([_P, _OUTF], f32, name="ex", tag="ex")
            nc.scalar.activation(ex[:], z[:], ACT.Exp)
            q = epool.tile([_P, _OUTF], f32, name="q", tag="q")
            nc.scalar.activation(q[:], ex[:], ACT.Relu, bias=1.0, scale=-1.0)
            nc.vector.tensor_scalar_max(z[:], z[:], 0.0)
            ob = out_stage[:, t * _OUTF:(t + 1) * _OUTF]
            nc.vector.tensor_sub(ob, z[:], q[:])
        nc.sync.dma_start(out[:, :], out_stage[:])

    nc.compile()
    return nc


def _spot_check(out, adj, x, W):
    """Validate a few output rows on host (guards against rare HW transients;
    ~4x the bf16 noise floor). Returns max relative error over the sample."""
    rows = np.arange(_NCORES * 16) * (_N // (_NCORES * 16)) + 7
    h = x.astype(np.float32) @ W.astype(np.float32)
    asel = adj[rows].astype(np.float32)
    s = (asel @ h) / asel.sum(axis=1, keepdims=True)
    want = np.where(s > 0, s, np.expm1(s))
    return np.abs(out[rows] - want).max() / max(np.abs(want).max(), 1e-6)


def kernel(adj, x, W, a=None):
    global _cached_nc, last_results
    from concurrent.futures import ThreadPoolExecutor

    from concourse.bass_utils import run_bass_kernel_spmd

    adj = np.ascontiguousarray(adj)
    xT = np.ascontiguousarray(np.asarray(x, dtype=np.float32).T)
    W = np.ascontiguousarray(np.asarray(W, dtype=np.float32))

    def shard(c):
        return np.ascontiguousarray(adj[c * _ROWS:(c + 1) * _ROWS, :].T)

    with ThreadPoolExecutor(_NCORES) as ex:
        shards = list(ex.map(shard, range(_NCORES)))

    if _cached_nc is None:
        _cached_nc = _build_nc()

    in_maps = [{"adjT": shards[c], "xT": xT, "W": W} for c in range(_NCORES)]
    out = None
    for _attempt in range(3):
        try:
            last_results = run_bass_kernel_spmd(
                _cached_nc, in_maps, core_ids=list(range(_NCORES))
            )
        except ModuleNotFoundError:
            # BASS_TRACE set but this image lacks the axon NTFF hook module;
            # rerun with tracing forced off
            import os

            os.environ["BASS_NEVER_TRACE"] = "1"
            last_results = run_bass_kernel_spmd(
                _cached_nc, in_maps, core_ids=list(range(_NCORES))
            )
        out = np.concatenate(
            [
                last_results.results[c]["out"]
                .reshape(_P, _ROWS // _P, _OUTF)
                .transpose(1, 0, 2)
                .reshape(_ROWS, _OUTF)
                for c in range(_NCORES)
            ],
            axis=0,
        ).astype(np.float32)
        if _spot_check(out, adj, x, W) < 1.5e-2:
            break
    return out



# revision 2
# speedup vs baseline: 2.4924x; 2.4924x over previous
"""GAT layer kernel for Trainium2, sharded across 8 NeuronCores.

Math: since adj is 0/1 and the attention logit e_i is constant across row i,
the masked softmax collapses to attention[i,j] = adj[i,j] / rowdeg(i), so

    out = elu((adj @ h) / d),   h = x @ W,   d = adj @ ones

Per-core strategy (core c owns destination rows R_c = [c*1536, (c+1)*1536)):
  - host passes adjT_c = adj[R_c, :].T packed as fp8e4m3 (0/1 are exact, so
    the pack is lossless; 4x less HBM traffic than the int32 original:
    18.9 MB instead of 75.5 MB per core)
  - host passes xT bf16 (the DMA cast the baseline did on-device anyway)
  - device computes full h once, augmented with a ones column
    -> h_aug [12288, 65] bf16 in SBUF
  - main loop over 48 k2-blocks: HWDGE pure-copy DMA of adjT [256, 1536] fp8,
    then PE accumulates s_aug^T[65, 1536] += h_aug[kb].T @ adjT (mixed-dtype
    matmul: bf16 stationary x fp8 moving)
  - epilogue: PE-transpose s_aug^T back to row-major blocks, divide by the
    degree column, apply ELU, store [1536, 64] f32.
The adj traffic (18.9 MB fp8 per core) is the memory roofline.
"""

import numpy as np

_N = 12288
_P = 128
_NCORES = 8
_ROWS = _N // _NCORES          # 1536 destination rows per core
_KB2 = _N // (2 * _P)          # 48 double k-blocks
_INF = 256
_OUTF = 64
_HA = _OUTF + 1                # h augmented with ones column
_HS = 80                       # h_aug block stride (padded)
_MT = _ROWS // 512             # 3 moving-operand tiles per k-block

_cached_nc = None
last_results = None            # BassKernelResults of the most recent run


def _build_nc():
    from contextlib import ExitStack

    import concourse.bacc as bacc
    import concourse.mybir as mybir
    import concourse.tile as tile
    from concourse.masks import make_identity

    f32 = mybir.dt.float32
    bf16 = mybir.dt.bfloat16
    f8 = mybir.dt.float8e4
    ACT = mybir.ActivationFunctionType

    nc = bacc.Bacc("TRN2", target_bir_lowering=False, debug=False)
    adjT = nc.dram_tensor("adjT", [_N, _ROWS], f8, kind="ExternalInput")
    xT = nc.dram_tensor("xT", [_INF, _N], bf16, kind="ExternalInput")
    W = nc.dram_tensor("W", [_INF, _OUTF], bf16, kind="ExternalInput")
    # raw staging layout [partition, t*64+f]; host reassembles rows as
    # out[t*128+p, f] = out_raw[p, t*64+f]. Keeps the store at 3KB/partition
    # contiguous chunks.
    out = nc.dram_tensor("out", [_P, (_ROWS // _P) * _OUTF], f32,
                         kind="ExternalOutput")

    with ExitStack() as ctx:
        tc = ctx.enter_context(tile.TileContext(nc))
        cpool = ctx.enter_context(tc.tile_pool(name="cpool", bufs=1))
        xpool = ctx.enter_context(tc.tile_pool(name="xpool", bufs=1))
        hpool = ctx.enter_context(tc.tile_pool(name="hpool", bufs=1))
        apool = ctx.enter_context(tc.tile_pool(name="apool", bufs=16))
        epool = ctx.enter_context(tc.tile_pool(name="epool", bufs=4))
        ps_main = ctx.enter_context(tc.tile_pool(name="ps_main", bufs=1, space="PSUM"))
        ps_h = ctx.enter_context(tc.tile_pool(name="ps_h", bufs=2, space="PSUM"))
        ps_t = ctx.enter_context(tc.tile_pool(name="ps_t", bufs=3, space="PSUM"))

        ident = cpool.tile([_P, _P], f32, name="ident", tag="ident")
        make_identity(nc, ident[:])

        w_sb = cpool.tile([_P, 2 * _OUTF], bf16, name="w_sb", tag="w_sb")
        nc.sync.dma_start(w_sb[:, 0:_OUTF], W[0:_P, :])
        nc.sync.dma_start(w_sb[:, _OUTF:], W[_P:, :])

        xt0 = xpool.tile([_P, _N], bf16, name="xt0", tag="xt0")
        nc.sync.dma_start(xt0[:], xT[0:_P, :])
        xt1 = xpool.tile([_P, _N], bf16, name="xt1", tag="xt1")
        nc.sync.dma_start(xt1[:], xT[_P:, :])

        # h_aug blocks: block ib occupies [_P, ib, 0:65] of a 3D tile with
        # stride-80 blocks; col 64 of each block is the ones column (memset
        # once, never rewritten)
        h_aug = hpool.tile([_P, 2 * _KB2, _HS], bf16, name="h_aug", tag="h_aug")
        nc.gpsimd.memset(h_aug[:], 1.0)

        # h-phase in groups of 4 k-blocks per PSUM tile to batch the
        # PSUM->SBUF activation copies
        for g in range(2 * _KB2 // 4):
            ph = ps_h.tile([_P, 4, _OUTF], f32, name="ph", tag="ph")
            for u in range(4):
                ib = 4 * g + u
                nc.tensor.matmul(ph[:, u, :], lhsT=xt0[:, ib * _P:(ib + 1) * _P],
                                 rhs=w_sb[:, 0:_OUTF], start=True, stop=False)
                nc.tensor.matmul(ph[:, u, :], lhsT=xt1[:, ib * _P:(ib + 1) * _P],
                                 rhs=w_sb[:, _OUTF:], start=False, stop=True)
            nc.scalar.activation(h_aug[:, 4 * g:4 * g + 4, 0:_OUTF], ph[:],
                                 ACT.Copy)

        # main accumulation: s_aug^T[f, m] += sum_kb h_aug[kb].T @ adjT[kb]
        # adj DMAs alternate between the two HWDGE queues (sync / scalar)
        ps = ps_main.tile([_HA, _ROWS], f32, name="ps", tag="ps")
        for kb2 in range(_KB2):
            at3 = apool.tile([_P, 2, _ROWS], f8, name="at", tag="at")
            eng = nc.sync if (kb2 % 2 == 0) else nc.scalar
            eng.dma_start(
                at3[:],
                adjT[kb2 * 2 * _P:(kb2 + 1) * 2 * _P, :].rearrange(
                    "(t p) j -> p t j", p=_P),
            )
            for t in range(2):
                for mt in range(_MT):
                    nc.tensor.matmul(
                        ps[:, mt * 512:(mt + 1) * 512],
                        lhsT=h_aug[:, kb2 * 2 + t, 0:_HA],
                        rhs=at3[:, t, mt * 512:(mt + 1) * 512],
                        start=(kb2 == 0 and t == 0),
                        stop=(kb2 == _KB2 - 1 and t == 1),
                    )

        # epilogue: transpose back, normalize by degree, ELU; stage all 12
        # row-blocks into one SBUF tile and store with a single DMA
        out_stage = hpool.tile([_P, (_ROWS // _P) * _OUTF], f32,
                               name="out_stage", tag="out_stage")
        for t in range(_ROWS // _P):
            sT = epool.tile([_HA, _P], f32, name="sT", tag="sT")
            nc.scalar.activation(sT[:], ps[:, t * _P:(t + 1) * _P], ACT.Copy)
            tp = ps_t.tile([_P, _HA], f32, name="tp", tag="tp")
            nc.tensor.transpose(tp[:], sT[:], ident[0:_HA, 0:_HA])
            rec = epool.tile([_P, 1], f32, name="rec", tag="rec")
            nc.vector.reciprocal(rec[:], tp[:, _OUTF:_HA])
            z = epool.tile([_P, _OUTF], f32, name="z", tag="z")
            nc.vector.tensor_scalar_mul(z[:], tp[:, 0:_OUTF], rec[:])
            # elu(z) = relu(z) - relu(1 - exp(z)): exact both branches
            ex = epool.tile([_P, _OUTF], f32, name="ex", tag="ex")
            nc.scalar.activation(ex[:], z[:], ACT.Exp)
            q = epool.tile([_P, _OUTF], f32, name="q", tag="q")
            nc.scalar.activation(q[:], ex[:], ACT.Relu, bias=1.0, scale=-1.0)
            nc.vector.tensor_scalar_max(z[:], z[:], 0.0)
            ob = out_stage[:, t * _OUTF:(t + 1) * _OUTF]
            nc.vector.tensor_sub(ob, z[:], q[:])
        nc.sync.dma_start(out[:, :], out_stage[:])

    nc.compile()
    return nc


def _spot_check(out, adj, x, W):
    """Validate a few output rows on host (guards against rare HW transients;
    ~4x the bf16 noise floor). Returns max relative error over the sample."""
    rows = np.arange(_NCORES * 16) * (_N // (_NCORES * 16)) + 7
    h = x.astype(np.float32) @ W.astype(np.float32)
    asel = adj[rows].astype(np.float32)
    s = (asel @ h) / asel.sum(axis=1, keepdims=True)
    want = np.where(s > 0, s, np.expm1(s))
    return np.abs(out[rows] - want).max() / max(np.abs(want).max(), 1e-6)


def kernel(adj, x, W, a=None):
    global _cached_nc, last_results
    from concurrent.futures import ThreadPoolExecutor

    import ml_dtypes

    from concourse.bass_utils import run_bass_kernel_spmd

    adj = np.ascontiguousarray(adj)
    xT = np.asarray(x, dtype=np.float32).T.astype(ml_dtypes.bfloat16)
    Wb = np.asarray(W, dtype=np.float32).astype(ml_dtypes.bfloat16)

    def shard(c):
        # adj values are 0/1; 0x38 is the fp8e4m3 bit pattern for 1.0, so
        # this pack is exact
        blk = adj[c * _ROWS:(c + 1) * _ROWS, :].T
        return (blk.astype(np.uint8) * np.uint8(0x38)).view(ml_dtypes.float8_e4m3)

    with ThreadPoolExecutor(_NCORES) as ex:
        shards = list(ex.map(shard, range(_NCORES)))

    if _cached_nc is None:
        _cached_nc = _build_nc()

    in_maps = [{"adjT": shards[c], "xT": xT, "W": Wb} for c in range(_NCORES)]
    out = None
    for _attempt in range(3):
        try:
            last_results = run_bass_kernel_spmd(
                _cached_nc, in_maps, core_ids=list(range(_NCORES))
            )
        except ModuleNotFoundError:
            # BASS_TRACE set but this image lacks the axon NTFF hook module;
            # rerun with tracing forced off
            import os

            os.environ["BASS_NEVER_TRACE"] = "1"
            last_results = run_bass_kernel_spmd(
                _cached_nc, in_maps, core_ids=list(range(_NCORES))
            )
        out = np.concatenate(
            [
                last_results.results[c]["out"]
                .reshape(_P, _ROWS // _P, _OUTF)
                .transpose(1, 0, 2)
                .reshape(_ROWS, _OUTF)
                for c in range(_NCORES)
            ],
            axis=0,
        ).astype(np.float32)
        if _spot_check(out, adj, x, W) < 1.5e-2:
            break
    return out
